# revision 2
# baseline (speedup 1.0000x reference)
"""DeepGSEA forward kernel for 8x Trainium2 NeuronCores (Bass/Tile).

Strategy: concept-parallel sharding. The masked-input GEMM
einsum('bcg,gh->bch', X[:,None,:]*M[None], W1) is computed as per-concept
gathered GEMMs: only ~10% of genes are active per concept, so the
contraction shrinks from G=2000 to S_pad (~256). Each of the 8 cores
owns ceil(C/8) concept slots (padded with dummy zero-concepts) and the
full batch.

Per core, per (256-row batch chunk, concept):
  stage A: h1 = lrelu(Xg @ W1g)      fp32r GEMMs, bias b1 folded in via a
                                      constant-1 row of XgT
  stage B: h2 = lrelu(h1 @ W2 + b2)  fp32r
  stage C: one fused GEMM produces [Z | 2*(Z.p)*gi] where the
           prototype-dot weights Qdot[h,kp] = sum_o cw[c,o,h]*proto[c,kp,o]
           and gi = 1/(2 exp(logvar)) are folded on the host.
  tail (DVE/ACT): -scaled = gi*(-|Z+cb|^2) + 2*dots*gi + cstg,
           c_logits = max_p(-scaled) (+c_bias folded into cstg),
           partial logits acc += sum_c exp(c_logits + log|imp_c|).

Host finishes with: gather c_logits, sum the 8 per-core exp-partials,
logits = log(sum + 1e-16) + bias.
"""
import os
import sys

if os.environ.get("JAX_PLATFORMS") and "axon" not in os.environ.get("JAX_PLATFORMS", ""):
    # kernel must run on the neuron cores; drop a cpu-only pin if present
    os.environ["JAX_PLATFORMS"] = ""
sys.path.insert(0, "/opt/trn_rl_repo")

import numpy as np
import concourse.bass as bass
import concourse.mybir as mybir
from concourse import bacc
from concourse import tile
from concourse import bass_utils

F32 = mybir.dt.float32
F32R = mybir.dt.float32r
AF = mybir.ActivationFunctionType
ALU = mybir.AluOpType
AXL = mybir.AxisListType

N_CORES = 8
LAST_RESULTS = None  # set by kernel(); test harness reads exec_time_ns


def _build_nc(CS, NSS, NBC, use_b2):
    """Build the per-core Bass program.

    CS: concept slots per core; NSS: 128-row contraction chunks of stage A;
    NBC: number of 256-column batch chunks; use_b2: emit per-partition bias
    adds in stage B's epilogue.
    """
    BP = NBC * 256  # padded batch
    KP = 40         # K*P prototype columns, p-major (p*10+k)
    ZD = 32
    WCW = ZD + KP   # 72: fused stage-C weight columns

    nc = bacc.Bacc("TRN2", target_bir_lowering=False, debug=False,
                   num_devices=N_CORES)

    xgt_d = nc.dram_tensor("xgt", (CS * NSS * 128, BP), F32R, kind="ExternalInput")
    w1g_d = nc.dram_tensor("w1g", (CS * NSS * 128, 256), F32R, kind="ExternalInput")
    w2_d = nc.dram_tensor("w2", (128, 512), F32R, kind="ExternalInput")
    wc_d = nc.dram_tensor("wc", (CS * 2 * 128, WCW), F32R, kind="ExternalInput")
    cbb_d = nc.dram_tensor("cbb", (128, CS * ZD), F32, kind="ExternalInput")
    gib_d = nc.dram_tensor("gib", (128, CS * KP), F32, kind="ExternalInput")
    cstg_d = nc.dram_tensor("cstg", (128, CS * KP), F32, kind="ExternalInput")
    limp_d = nc.dram_tensor("limp", (128, CS * 10), F32, kind="ExternalInput")
    if use_b2:
        b2b_d = nc.dram_tensor("b2b", (128, 2), F32, kind="ExternalInput")

    clout_d = nc.dram_tensor("clout", (128, NBC * 2 * CS * 10), F32,
                             kind="ExternalOutput")
    acc_d = nc.dram_tensor("acc", (128, NBC * 2 * 10), F32, kind="ExternalOutput")

    with tile.TileContext(nc) as tc:
        with tc.tile_pool(name="persist", bufs=1) as pp, \
             tc.tile_pool(name="h1p", bufs=2) as pH1, \
             tc.tile_pool(name="h2p", bufs=2) as pH2, \
             tc.tile_pool(name="tails", bufs=2) as pT, \
             tc.tile_pool(name="pa", bufs=2, space="PSUM") as pA, \
             tc.tile_pool(name="pb", bufs=2, space="PSUM") as pB, \
             tc.tile_pool(name="pc", bufs=2, space="PSUM") as pC:

            # ---- persistent inputs ----
            xgt_t = []
            w1g_t = []
            wc_t = []
            for c in range(CS):
                w1 = pp.tile([128, NSS * 256], F32R, tag=f"w1g{c}")
                for ss in range(NSS):
                    r0 = (c * NSS + ss) * 128
                    nc.sync.dma_start(w1[:, ss * 256:(ss + 1) * 256],
                                      w1g_d.ap()[r0:r0 + 128, :])
                w1g_t.append(w1)
                wc = pp.tile([128, 2 * WCW], F32R, tag=f"wc{c}")
                for kk in range(2):
                    r0 = (c * 2 + kk) * 128
                    nc.sync.dma_start(wc[:, kk * WCW:(kk + 1) * WCW],
                                      wc_d.ap()[r0:r0 + 128, :])
                wc_t.append(wc)
                xs = []
                for ss in range(NSS):
                    xt = pp.tile([128, BP], F32R, tag=f"xgt{c}_{ss}")
                    r0 = (c * NSS + ss) * 128
                    nc.sync.dma_start(xt[:], xgt_d.ap()[r0:r0 + 128, :])
                    xs.append(xt)
                xgt_t.append(xs)

            w2_t = pp.tile([128, 512], F32R, tag="w2")
            nc.sync.dma_start(w2_t[:], w2_d.ap())
            cbb_t = pp.tile([128, CS * ZD], F32, tag="cbb")
            nc.sync.dma_start(cbb_t[:], cbb_d.ap())
            gib_t = pp.tile([128, CS * KP], F32, tag="gib")
            nc.sync.dma_start(gib_t[:], gib_d.ap())
            cstg_t = pp.tile([128, CS * KP], F32, tag="cstg")
            nc.sync.dma_start(cstg_t[:], cstg_d.ap())
            limp_t = pp.tile([128, CS * 10], F32, tag="limp")
            nc.sync.dma_start(limp_t[:], limp_d.ap())
            if use_b2:
                b2b_t = pp.tile([128, 2], F32, tag="b2b")
                nc.sync.dma_start(b2b_t[:], b2b_d.ap())

            clout_t = pp.tile([128, NBC * 2 * CS * 10], F32, tag="clout")
            acc_t = pp.tile([128, NBC * 2 * 10], F32, tag="accout")

            # ---- main loop ----
            for bc in range(NBC):
                b0 = bc * 256
                psC = [pC.tile([128, CS * WCW], F32, tag=f"psc{b}", name=f"psc{b}")
                       for b in range(2)]
                for c in range(CS):
                    # stage A
                    psA = pA.tile([128, 512], F32, tag="psa")
                    for hh in range(2):
                        for ss in range(NSS):
                            nc.tensor.matmul(
                                psA[:, hh * 256:(hh + 1) * 256],
                                w1g_t[c][:, ss * 256 + hh * 128:ss * 256 + hh * 128 + 128],
                                xgt_t[c][ss][:, b0:b0 + 256],
                                start=(ss == 0), stop=(ss == NSS - 1))
                    h1 = pH1.tile([128, 512], F32R, tag="h1")
                    nc.scalar.activation(h1[:], psA[:], AF.Lrelu,
                                         bias=0.0, scale=1.0, alpha=0.01)
                    # stage B
                    psB = pB.tile([128, 512], F32, tag="psb")
                    for kk in range(2):
                        for hh in range(2):
                            nc.tensor.matmul(
                                psB[:, kk * 256:(kk + 1) * 256],
                                w2_t[:, hh * 256 + kk * 128:hh * 256 + kk * 128 + 128],
                                h1[:, hh * 256:(hh + 1) * 256],
                                start=(hh == 0), stop=(hh == 1))
                    h2 = pH2.tile([128, 512], F32R, tag="h2")
                    if use_b2:
                        for kk in range(2):
                            nc.scalar.activation(
                                h2[:, kk * 256:(kk + 1) * 256],
                                psB[:, kk * 256:(kk + 1) * 256],
                                AF.Lrelu, bias=b2b_t[:, kk:kk + 1],
                                scale=1.0, alpha=0.01)
                    else:
                        nc.scalar.activation(h2[:], psB[:], AF.Lrelu,
                                             bias=0.0, scale=1.0, alpha=0.01)
                    # stage C (fused Z + prototype dots)
                    for bsub in range(2):
                        for kk in range(2):
                            nc.tensor.matmul(
                                psC[bsub][:, c * WCW:(c + 1) * WCW],
                                h2[:, kk * 256 + bsub * 128:kk * 256 + bsub * 128 + 128],
                                wc_t[c][:, kk * WCW:(kk + 1) * WCW],
                                start=(kk == 0), stop=(kk == 1),
                                skip_group_check=True)

                # ---- tail: per 128-row batch sub-chunk ----
                for bsub in range(2):
                    psP = psC[bsub]
                    pv = psP[:].rearrange("p (c w) -> p c w", c=CS, w=WCW)
                    zv = pv[:, :, 0:ZD]
                    dv = pv[:, :, ZD:WCW]
                    t1 = pT.tile([128, CS * ZD], F32, tag="t1")
                    t1v = t1[:].rearrange("p (c w) -> p c w", c=CS, w=ZD)
                    cbv = cbb_t[:].rearrange("p (c w) -> p c w", c=CS, w=ZD)
                    nc.vector.tensor_tensor(t1v, zv, cbv, op=ALU.add)
                    sq = pT.tile([128, CS * ZD], F32, tag="sq")
                    nc.vector.tensor_tensor(sq[:], t1[:], t1[:], op=ALU.mult)
                    nzz = pT.tile([128, CS], F32, tag="nzz")
                    nc.vector.tensor_reduce(
                        nzz[:], sq[:].rearrange("p (c w) -> p c w", c=CS, w=ZD),
                        axis=AXL.X, op=ALU.add, negate=True)
                    m1 = pT.tile([128, CS * KP], F32, tag="m1")
                    nzz_b = nzz[:].unsqueeze(-1).broadcast_to((128, CS, KP))
                    nc.vector.tensor_tensor(
                        m1[:].rearrange("p (c w) -> p c w", c=CS, w=KP),
                        gib_t[:].rearrange("p (c w) -> p c w", c=CS, w=KP),
                        nzz_b, op=ALU.mult)
                    m2 = pT.tile([128, CS * KP], F32, tag="m2")
                    nc.vector.tensor_tensor(
                        m2[:].rearrange("p (c w) -> p c w", c=CS, w=KP),
                        m1[:].rearrange("p (c w) -> p c w", c=CS, w=KP),
                        dv, op=ALU.add)
                    m3 = pT.tile([128, CS * KP], F32, tag="m3")
                    nc.vector.tensor_tensor(m3[:], m2[:], cstg_t[:], op=ALU.add)
                    # max over p (columns c*40 + p*10 + k, p-major)
                    m3v = m3[:].rearrange("p (c q k) -> p c q k", c=CS, q=4, k=10)
                    m4a = pT.tile([128, CS * 10], F32, tag="m4a")
                    m4b = pT.tile([128, CS * 10], F32, tag="m4b")
                    m4av = m4a[:].rearrange("p (c k) -> p c k", c=CS, k=10)
                    m4bv = m4b[:].rearrange("p (c k) -> p c k", c=CS, k=10)
                    nc.vector.tensor_tensor(m4av, m3v[:, :, 0, :], m3v[:, :, 1, :],
                                            op=ALU.max)
                    nc.vector.tensor_tensor(m4bv, m3v[:, :, 2, :], m3v[:, :, 3, :],
                                            op=ALU.max)
                    coff = (bc * 2 + bsub) * CS * 10
                    clsl = clout_t[:, coff:coff + CS * 10]
                    nc.vector.tensor_tensor(clsl, m4a[:], m4b[:], op=ALU.max)
                    # exp path
                    einp = pT.tile([128, CS * 10], F32, tag="einp")
                    nc.vector.tensor_tensor(einp[:], clsl, limp_t[:], op=ALU.add)
                    e = pT.tile([128, 10 * CS], F32, tag="e")
                    e_tw = e[:].rearrange("p (k c) -> p c k", c=CS, k=10)
                    nc.scalar.activation(e_tw, einp[:], AF.Exp, bias=0.0, scale=1.0)
                    aoff = (bc * 2 + bsub) * 10
                    nc.vector.tensor_reduce(
                        acc_t[:, aoff:aoff + 10],
                        e[:].rearrange("p (k c) -> p k c", k=10, c=CS),
                        axis=AXL.X, op=ALU.add)

            nc.sync.dma_start(clout_d.ap(), clout_t[:])
            nc.sync.dma_start(acc_d.ap(), acc_t[:])

    nc.compile()
    return nc


def kernel(X, M, W1, b1, W2, b2, cw, cb, prototypes, logvar, c_bias, bias,
           importance):
    global LAST_RESULTS
    X = np.ascontiguousarray(np.asarray(X, dtype=np.float32))
    M = np.asarray(M, dtype=np.float32)
    W1 = np.asarray(W1, dtype=np.float32)
    b1 = np.asarray(b1, dtype=np.float32)
    W2 = np.asarray(W2, dtype=np.float32)
    b2 = np.asarray(b2, dtype=np.float32)
    cw = np.asarray(cw, dtype=np.float32)
    cb = np.asarray(cb, dtype=np.float32)
    prototypes = np.asarray(prototypes, dtype=np.float32)
    logvar = np.asarray(logvar, dtype=np.float32)
    c_bias = np.asarray(c_bias, dtype=np.float32)
    bias = np.asarray(bias, dtype=np.float32)
    importance = np.asarray(importance, dtype=np.float32)

    B, G = X.shape
    C = M.shape[0]
    H = W1.shape[1]
    ZD = cw.shape[1]
    K, P = prototypes.shape[1], prototypes.shape[2]
    assert H == 256 and ZD == 32 and K == 10 and P == 4

    # ---- sharding: concepts -> cores (contiguous blocks) ----
    base, rem = divmod(C, N_CORES)
    counts = [base + 1 if i < rem else base for i in range(N_CORES)]
    starts = np.concatenate([[0], np.cumsum(counts)])
    CS = max(counts)

    BP = ((B + 255) // 256) * 256
    NBC = BP // 256
    Xp = X
    if BP != B:
        Xp = np.zeros((BP, G), np.float32)
        Xp[:B] = X

    nnz = [np.nonzero(M[c])[0] for c in range(C)]
    S_pad = max(2, max(len(z) for z in nnz) + 1)
    S_pad = ((S_pad + 127) // 128) * 128
    NSS = S_pad // 128

    KPn = K * P
    # p-major prototype ordering: kp = p*K + k
    protf = prototypes.transpose(0, 2, 1, 3).reshape(C, KPn, ZD)
    gi = 1.0 / (2.0 * np.exp(logvar.transpose(0, 2, 1).reshape(C, KPn)))
    PP = np.sum(protf * protf, axis=2)                     # [C, KP]
    cbdot = np.einsum("cpo,co->cp", protf, cb[0])          # [C, KP]
    cstg = -(PP - 2.0 * cbdot) * gi + np.tile(c_bias, (1, P))  # [C, KP] (+c_bias)
    imp = np.abs(importance[0])                            # [C]
    limp = np.maximum(np.log(np.maximum(imp, 1e-300)), -80.0)

    use_b2 = bool(np.any(b2 != 0.0))

    XT = np.ascontiguousarray(Xp.T)  # [G, BP]

    in_maps = []
    for i in range(N_CORES):
        cs, ce = starts[i], starts[i + 1]
        xgt = np.zeros((CS * NSS * 128, BP), np.float32)
        w1g = np.zeros((CS * NSS * 128, 256), np.float32)
        wc = np.zeros((CS * 2 * 128, ZD + KPn), np.float32)
        cbb = np.zeros((128, CS * ZD), np.float32)
        gib = np.ones((128, CS * KPn), np.float32)
        cstgb = np.zeros((128, CS * KPn), np.float32)
        limpb = np.full((128, CS * K), -80.0, np.float32)
        for j in range(ce - cs):
            c = cs + j
            idx = nnz[c]
            n = len(idx)
            r0 = j * NSS * 128
            xgt[r0:r0 + n] = XT[idx]
            xgt[r0 + n] = 1.0  # bias row
            w1g[r0:r0 + n] = W1[idx] * M[c, idx][:, None]
            w1g[r0 + n] = b1
            # fused stage-C weights
            Qdot = np.einsum("oh,po->hp", cw[c], protf[c])  # [H, KP]
            wcc = np.concatenate(
                [cw[c].T, 2.0 * Qdot * gi[c][None, :]], axis=1)  # [H, 72]
            wc[j * 256:(j + 1) * 256] = wcc
            cbb[:, j * ZD:(j + 1) * ZD] = cb[0, c][None, :]
            gib[:, j * KPn:(j + 1) * KPn] = gi[c][None, :]
            cstgb[:, j * KPn:(j + 1) * KPn] = cstg[c][None, :]
            limpb[:, j * K:(j + 1) * K] = limp[c]
        m = {"xgt": xgt, "w1g": w1g,
             "w2": W2.reshape(2, 128, 256).transpose(1, 0, 2).reshape(128, 512),
             "wc": wc, "cbb": cbb, "gib": gib, "cstg": cstgb, "limp": limpb}
        if use_b2:
            m["b2b"] = b2.reshape(2, 128).T.copy()
        in_maps.append(m)

    nc = _build_nc(CS, NSS, NBC, use_b2)
    res = bass_utils.run_bass_kernel_spmd(nc, in_maps, core_ids=list(range(N_CORES)))
    LAST_RESULTS = res

    # ---- unshard ----
    c_logits = np.zeros((B, C, K), np.float32)
    total = np.zeros((BP, K), np.float64)
    for i in range(N_CORES):
        out = res.results[i]
        cl = out["clout"].reshape(128, NBC * 2, CS, K).transpose(1, 0, 2, 3)
        cl = cl.reshape(BP, CS, K)
        cs, ce = starts[i], starts[i + 1]
        c_logits[:, cs:ce, :] = cl[:B, :ce - cs, :]
        ac = out["acc"].reshape(128, NBC * 2, K).transpose(1, 0, 2).reshape(BP, K)
        total += ac
    logits = (np.log(total[:B] + 1e-16) + bias[None, :]).astype(np.float32)
    return logits, c_logits


# revision 5
# speedup vs baseline: 1.5939x; 1.5939x over previous
"""DeepGSEA forward kernel for 8x Trainium2 NeuronCores (Bass/Tile).

Strategy: concept-parallel sharding. The masked-input GEMM
einsum('bcg,gh->bch', X[:,None,:]*M[None], W1) is computed as per-concept
gathered GEMMs: only ~10% of genes are active per concept, so the
contraction shrinks from G=2000 to S_pad (~256). Each of the 8 cores
owns ceil(C/8) concept slots (padded with dummy zero-concepts) and the
full batch.

Per core, per (256-row batch chunk bc, concept c) — software-pipelined
with a 2-deep stage skew so PE never waits on ACT epilogues:
  stage A: h1 = lrelu(Xg @ W1g)      fp32r GEMMs, b1 folded via a
                                      constant-1 row of XgT
  stage B: h2 = lrelu(h1 @ W2 + b2)  fp32r; h2 stored bf16
  stage C: bf16 fused GEMM -> [Z | 2*(Z.proto)*gi], gi = 1/(2 e^logvar),
           Qdot[h,kp] = sum_o cw[c,o,h]*proto[c,kp,o] folded on host.
  tail (DVE, once per bc over both 128-row halves):
           -scaled = gi*(-|Z+cb|^2) + 2*dots*gi + cstg,
           c_logits = max_p(-scaled)   (c_bias folded into cstg),
           einp = c_logits + log|imp_c|
  end phase (single ACT table load): e = exp(einp) written k-major,
           acc[b,k] = sum_c e.

Host finishes: gather c_logits, sum per-core exp-partials,
logits = log(sum + 1e-16) + bias.
"""
import os
import sys

if os.environ.get("JAX_PLATFORMS") and "axon" not in os.environ.get("JAX_PLATFORMS", ""):
    os.environ["JAX_PLATFORMS"] = ""
sys.path.insert(0, "/opt/trn_rl_repo")

import numpy as np
import ml_dtypes
import concourse.bass as bass
import concourse.mybir as mybir
from concourse import bacc
from concourse import tile
from concourse import bass_utils

F32 = mybir.dt.float32
F32R = mybir.dt.float32r
BF16 = mybir.dt.bfloat16
AF = mybir.ActivationFunctionType
ALU = mybir.AluOpType
AXL = mybir.AxisListType

N_CORES = 8
LAST_RESULTS = None  # set by kernel(); test harness reads exec_time_ns


def _build_nc(CS, NSS, NBC, use_b2):
    BP = NBC * 256
    KP = 40
    ZD = 32
    WCW = ZD + KP  # 72

    nc = bacc.Bacc("TRN2", target_bir_lowering=False, debug=False,
                   num_devices=N_CORES)

    xgt_d = nc.dram_tensor("xgt", (CS * NSS * 128, BP), F32R, kind="ExternalInput")
    w1g_d = nc.dram_tensor("w1g", (CS * NSS * 128, 256), F32R, kind="ExternalInput")
    w2_d = nc.dram_tensor("w2", (128, 512), F32R, kind="ExternalInput")
    wc_d = nc.dram_tensor("wc", (CS * 2 * 128, WCW), BF16, kind="ExternalInput")
    cbb_d = nc.dram_tensor("cbb", (128, CS * ZD), F32, kind="ExternalInput")
    gib_d = nc.dram_tensor("gib", (128, CS * KP), F32, kind="ExternalInput")
    cstg_d = nc.dram_tensor("cstg", (128, CS * KP), F32, kind="ExternalInput")
    limp_d = nc.dram_tensor("limp", (128, CS * 10), F32, kind="ExternalInput")
    if use_b2:
        b2b_d = nc.dram_tensor("b2b", (128, 2), F32, kind="ExternalInput")

    clout_d = nc.dram_tensor("clout", (128, NBC * 2 * CS * 10), F32,
                             kind="ExternalOutput")
    acc_d = nc.dram_tensor("acc", (128, NBC * 2 * 10), F32, kind="ExternalOutput")

    NITER = NBC * CS

    with tile.TileContext(nc) as tc:
        with tc.tile_pool(name="persist", bufs=1) as pp, \
             tc.tile_pool(name="h1p", bufs=2) as pH1, \
             tc.tile_pool(name="h2p", bufs=2) as pH2, \
             tc.tile_pool(name="tails", bufs=2) as pT, \
             tc.tile_pool(name="pa", bufs=2, space="PSUM") as pA, \
             tc.tile_pool(name="pb", bufs=2, space="PSUM") as pB, \
             tc.tile_pool(name="pc", bufs=2, space="PSUM") as pC:

            # ---- persistent inputs ----
            xgt_t = []
            w1g_t = []
            wc_t = []
            for c in range(CS):
                w1 = pp.tile([128, NSS * 256], F32R, tag=f"w1g{c}", name=f"w1g{c}")
                for ss in range(NSS):
                    r0 = (c * NSS + ss) * 128
                    nc.sync.dma_start(w1[:, ss * 256:(ss + 1) * 256],
                                      w1g_d.ap()[r0:r0 + 128, :])
                w1g_t.append(w1)
                wc = pp.tile([128, 2 * WCW], BF16, tag=f"wc{c}", name=f"wc{c}")
                for kk in range(2):
                    r0 = (c * 2 + kk) * 128
                    nc.sync.dma_start(wc[:, kk * WCW:(kk + 1) * WCW],
                                      wc_d.ap()[r0:r0 + 128, :])
                wc_t.append(wc)
                xs = []
                for ss in range(NSS):
                    xt = pp.tile([128, BP], F32R, tag=f"xgt{c}_{ss}",
                                 name=f"xgt{c}_{ss}")
                    r0 = (c * NSS + ss) * 128
                    nc.sync.dma_start(xt[:], xgt_d.ap()[r0:r0 + 128, :])
                    xs.append(xt)
                xgt_t.append(xs)

            w2_t = pp.tile([128, 512], F32R, tag="w2")
            nc.sync.dma_start(w2_t[:], w2_d.ap())
            cbb_t = pp.tile([128, CS * ZD], F32, tag="cbb")
            nc.sync.dma_start(cbb_t[:], cbb_d.ap())
            gib_t = pp.tile([128, CS * KP], F32, tag="gib")
            nc.sync.dma_start(gib_t[:], gib_d.ap())
            cstg_t = pp.tile([128, CS * KP], F32, tag="cstg")
            nc.sync.dma_start(cstg_t[:], cstg_d.ap())
            limp_t = pp.tile([128, CS * 10], F32, tag="limp")
            nc.sync.dma_start(limp_t[:], limp_d.ap())
            if use_b2:
                b2b_t = pp.tile([128, 2], F32, tag="b2b")
                nc.sync.dma_start(b2b_t[:], b2b_d.ap())

            clout_t = pp.tile([128, NBC * 2 * CS * 10], F32, tag="clout")
            einp_t = pp.tile([128, NBC * 2 * CS * 10], F32, tag="einp")
            acc_t = pp.tile([128, NBC * 2 * 10], F32, tag="accout")

            psA_m = {}
            h1_m = {}
            psB_m = {}
            h2_m = {}
            psC_m = {}

            def emit_A(i):
                bc, c = divmod(i, CS)
                b0 = bc * 256
                psA = pA.tile([128, 512], F32, tag="psa", name=f"psa{i}")
                for hh in range(2):
                    for ss in range(NSS):
                        nc.tensor.matmul(
                            psA[:, hh * 256:(hh + 1) * 256],
                            w1g_t[c][:, ss * 256 + hh * 128:ss * 256 + hh * 128 + 128],
                            xgt_t[c][ss][:, b0:b0 + 256],
                            start=(ss == 0), stop=(ss == NSS - 1))
                psA_m[i] = psA

            def emit_epiA(i):
                psA = psA_m.pop(i)
                h1 = pH1.tile([128, 512], F32R, tag="h1", name=f"h1_{i}")
                nc.scalar.activation(h1[:], psA[:], AF.Lrelu,
                                     bias=0.0, scale=1.0, alpha=0.01)
                h1_m[i] = h1

            def emit_B(i):
                h1 = h1_m.pop(i)
                psB = pB.tile([128, 512], F32, tag="psb", name=f"psb{i}")
                for kk in range(2):
                    for hh in range(2):
                        nc.tensor.matmul(
                            psB[:, kk * 256:(kk + 1) * 256],
                            w2_t[:, hh * 256 + kk * 128:hh * 256 + kk * 128 + 128],
                            h1[:, hh * 256:(hh + 1) * 256],
                            start=(hh == 0), stop=(hh == 1))
                psB_m[i] = psB

            def emit_epiB(i):
                psB = psB_m.pop(i)
                h2 = pH2.tile([128, 512], BF16, tag="h2", name=f"h2_{i}")
                if use_b2:
                    for kk in range(2):
                        nc.scalar.activation(
                            h2[:, kk * 256:(kk + 1) * 256],
                            psB[:, kk * 256:(kk + 1) * 256],
                            AF.Lrelu, bias=b2b_t[:, kk:kk + 1],
                            scale=1.0, alpha=0.01)
                else:
                    nc.scalar.activation(h2[:], psB[:], AF.Lrelu,
                                         bias=0.0, scale=1.0, alpha=0.01)
                h2_m[i] = h2

            def emit_C(i):
                bc, c = divmod(i, CS)
                h2 = h2_m.pop(i)
                if bc not in psC_m:
                    psC_m[bc] = pC.tile([128, 1024], F32, tag="psc",
                                        name=f"psc{bc}")
                psC = psC_m[bc]
                for bsub in range(2):
                    for kk in range(2):
                        nc.tensor.matmul(
                            psC[:, bsub * 512 + c * WCW:bsub * 512 + (c + 1) * WCW],
                            h2[:, kk * 256 + bsub * 128:kk * 256 + bsub * 128 + 128],
                            wc_t[c][:, kk * WCW:(kk + 1) * WCW],
                            start=(kk == 0), stop=(kk == 1),
                            skip_group_check=True)

            def emit_tail(bc):
                psC = psC_m.pop(bc)
                # views: cols = s*512 + c*72 + w
                pv = psC[:].rearrange("p (s r) -> p s r", s=2, r=512)
                pv = pv[:, :, 0:CS * WCW].rearrange(
                    "p s (c w) -> p s c w", c=CS, w=WCW)
                zv = pv[:, :, :, 0:ZD]
                dv = pv[:, :, :, ZD:WCW]
                cb_b = cbb_t[:].rearrange("p (c w) -> p c w", c=CS, w=ZD) \
                    .unsqueeze(1).broadcast_to((128, 2, CS, ZD))
                t1 = pT.tile([128, 2 * CS * ZD], F32, tag="t1", name=f"t1_{bc}")
                t1v = t1[:].rearrange("p (s c w) -> p s c w", s=2, c=CS, w=ZD)
                nc.vector.tensor_tensor(t1v, zv, cb_b, op=ALU.add)
                sq = pT.tile([128, 2 * CS * ZD], F32, tag="sq", name=f"sq_{bc}")
                nc.vector.tensor_tensor(sq[:], t1[:], t1[:], op=ALU.mult)
                nzz = pT.tile([128, 2 * CS], F32, tag="nzz", name=f"nzz_{bc}")
                nc.vector.tensor_reduce(
                    nzz[:], sq[:].rearrange("p (s c w) -> p s c w",
                                            s=2, c=CS, w=ZD),
                    axis=AXL.X, op=ALU.add, negate=True)
                gi_b = gib_t[:].rearrange("p (c w) -> p c w", c=CS, w=KP) \
                    .unsqueeze(1).broadcast_to((128, 2, CS, KP))
                nzz_b = nzz[:].rearrange("p (s c) -> p s c", s=2, c=CS) \
                    .unsqueeze(-1).broadcast_to((128, 2, CS, KP))
                m1 = pT.tile([128, 2 * CS * KP], F32, tag="m1", name=f"m1_{bc}")
                m1v = m1[:].rearrange("p (s c w) -> p s c w", s=2, c=CS, w=KP)
                nc.vector.tensor_tensor(m1v, gi_b, nzz_b, op=ALU.mult)
                m2 = pT.tile([128, 2 * CS * KP], F32, tag="m2", name=f"m2_{bc}")
                m2v = m2[:].rearrange("p (s c w) -> p s c w", s=2, c=CS, w=KP)
                nc.vector.tensor_tensor(m2v, m1v, dv, op=ALU.add)
                cst_b = cstg_t[:].rearrange("p (c w) -> p c w", c=CS, w=KP) \
                    .unsqueeze(1).broadcast_to((128, 2, CS, KP))
                m3 = pT.tile([128, 2 * CS * KP], F32, tag="m3", name=f"m3_{bc}")
                m3v = m3[:].rearrange("p (s c w) -> p s c w", s=2, c=CS, w=KP)
                nc.vector.tensor_tensor(m3v, m2v, cst_b, op=ALU.add)
                # max over the 4 prototype parts (cols p-major: kp = q*10+k)
                m5 = m3[:].rearrange("p (s c q k) -> p s c q k",
                                     s=2, c=CS, q=4, k=10)
                m4a = pT.tile([128, 2 * CS * 10], F32, tag="m4a", name=f"m4a_{bc}")
                m4b = pT.tile([128, 2 * CS * 10], F32, tag="m4b", name=f"m4b_{bc}")
                v_a = m4a[:].rearrange("p (s c k) -> p s c k", s=2, c=CS, k=10)
                v_b = m4b[:].rearrange("p (s c k) -> p s c k", s=2, c=CS, k=10)
                nc.vector.tensor_tensor(v_a, m5[:, :, :, 0, :], m5[:, :, :, 1, :],
                                        op=ALU.max)
                nc.vector.tensor_tensor(v_b, m5[:, :, :, 2, :], m5[:, :, :, 3, :],
                                        op=ALU.max)
                coff = bc * 2 * CS * 10
                clsl = clout_t[:, coff:coff + 2 * CS * 10]
                nc.vector.tensor_tensor(clsl, m4a[:], m4b[:], op=ALU.max)
                li_b = limp_t[:].rearrange("p (c k) -> p c k", c=CS, k=10) \
                    .unsqueeze(1).broadcast_to((128, 2, CS, 10))
                esl = einp_t[:, coff:coff + 2 * CS * 10]
                nc.vector.tensor_tensor(
                    esl.rearrange("p (s c k) -> p s c k", s=2, c=CS, k=10),
                    clsl.rearrange("p (s c k) -> p s c k", s=2, c=CS, k=10),
                    li_b, op=ALU.add)

            # stage skew: epiA lags A by 1 tick, B by 2, epiB by 3, C by 4 —
            # every cross-engine dependency crosses a tick boundary, so no
            # engine waits intra-tick on another.
            for i in range(NITER + 4):
                if 1 <= i and i - 1 < NITER:
                    emit_epiA(i - 1)
                if 3 <= i and i - 3 < NITER:
                    emit_epiB(i - 3)
                if i < NITER:
                    emit_A(i)
                if 2 <= i and i - 2 < NITER:
                    emit_B(i - 2)
                if 4 <= i and i - 4 < NITER:
                    emit_C(i - 4)
                    bc, c = divmod(i - 4, CS)
                    if c == CS - 1:
                        emit_tail(bc)

            # ---- end phase: exp + per-concept sum (one ACT table switch) ----
            for g in range(NBC):
                goff = g * 2 * CS * 10
                e = pT.tile([128, 2 * 10 * CS], F32, tag="e", name=f"e_{g}")
                # write transposed: input cols (s,c,k) -> e cols s*70 + k*7 + c
                e_tw = e[:].rearrange("p (s k c) -> p s c k", s=2, c=CS, k=10)
                nc.scalar.activation(e_tw, einp_t[:, goff:goff + 2 * CS * 10],
                                     AF.Exp, bias=0.0, scale=1.0)
                nc.vector.tensor_reduce(
                    acc_t[:, g * 20:(g + 1) * 20],
                    e[:].rearrange("p (s k c) -> p s k c", s=2, k=10, c=CS),
                    axis=AXL.X, op=ALU.add)

            nc.sync.dma_start(clout_d.ap(), clout_t[:])
            nc.sync.dma_start(acc_d.ap(), acc_t[:])

    nc.compile()
    return nc


def kernel(X, M, W1, b1, W2, b2, cw, cb, prototypes, logvar, c_bias, bias,
           importance):
    global LAST_RESULTS
    X = np.ascontiguousarray(np.asarray(X, dtype=np.float32))
    M = np.asarray(M, dtype=np.float32)
    W1 = np.asarray(W1, dtype=np.float32)
    b1 = np.asarray(b1, dtype=np.float32)
    W2 = np.asarray(W2, dtype=np.float32)
    b2 = np.asarray(b2, dtype=np.float32)
    cw = np.asarray(cw, dtype=np.float32)
    cb = np.asarray(cb, dtype=np.float32)
    prototypes = np.asarray(prototypes, dtype=np.float32)
    logvar = np.asarray(logvar, dtype=np.float32)
    c_bias = np.asarray(c_bias, dtype=np.float32)
    bias = np.asarray(bias, dtype=np.float32)
    importance = np.asarray(importance, dtype=np.float32)

    B, G = X.shape
    C = M.shape[0]
    H = W1.shape[1]
    ZD = cw.shape[1]
    K, P = prototypes.shape[1], prototypes.shape[2]
    assert H == 256 and ZD == 32 and K == 10 and P == 4

    base, rem = divmod(C, N_CORES)
    counts = [base + 1 if i < rem else base for i in range(N_CORES)]
    starts = np.concatenate([[0], np.cumsum(counts)])
    CS = max(counts)

    BP = ((B + 255) // 256) * 256
    NBC = BP // 256
    Xp = X
    if BP != B:
        Xp = np.zeros((BP, G), np.float32)
        Xp[:B] = X

    nnz = [np.nonzero(M[c])[0] for c in range(C)]
    S_pad = max(2, max(len(z) for z in nnz) + 1)
    S_pad = ((S_pad + 127) // 128) * 128
    NSS = S_pad // 128

    KPn = K * P
    protf = prototypes.transpose(0, 2, 1, 3).reshape(C, KPn, ZD)  # p-major kp
    gi = 1.0 / (2.0 * np.exp(logvar.transpose(0, 2, 1).reshape(C, KPn)))
    PP = np.sum(protf * protf, axis=2)
    cbdot = np.einsum("cpo,co->cp", protf, cb[0])
    cstg = -(PP - 2.0 * cbdot) * gi + np.tile(c_bias, (1, P))
    imp = np.abs(importance[0])
    limp = np.maximum(np.log(np.maximum(imp, 1e-300)), -80.0)

    use_b2 = bool(np.any(b2 != 0.0))

    XT = np.ascontiguousarray(Xp.T)

    in_maps = []
    for i in range(N_CORES):
        cs, ce = starts[i], starts[i + 1]
        xgt = np.zeros((CS * NSS * 128, BP), np.float32)
        w1g = np.zeros((CS * NSS * 128, 256), np.float32)
        wc = np.zeros((CS * 2 * 128, ZD + KPn), np.float32)
        cbb = np.zeros((128, CS * ZD), np.float32)
        gib = np.ones((128, CS * KPn), np.float32)
        cstgb = np.zeros((128, CS * KPn), np.float32)
        limpb = np.full((128, CS * K), -80.0, np.float32)
        for j in range(ce - cs):
            c = cs + j
            idx = nnz[c]
            n = len(idx)
            r0 = j * NSS * 128
            xgt[r0:r0 + n] = XT[idx]
            xgt[r0 + n] = 1.0
            w1g[r0:r0 + n] = W1[idx] * M[c, idx][:, None]
            w1g[r0 + n] = b1
            Qdot = np.einsum("oh,po->hp", cw[c], protf[c])
            wcc = np.concatenate(
                [cw[c].T, 2.0 * Qdot * gi[c][None, :]], axis=1)
            wc[j * 256:(j + 1) * 256] = wcc
            cbb[:, j * ZD:(j + 1) * ZD] = cb[0, c][None, :]
            gib[:, j * KPn:(j + 1) * KPn] = gi[c][None, :]
            cstgb[:, j * KPn:(j + 1) * KPn] = cstg[c][None, :]
            limpb[:, j * K:(j + 1) * K] = limp[c]
        m = {"xgt": xgt, "w1g": w1g,
             "w2": W2.reshape(2, 128, 256).transpose(1, 0, 2).reshape(128, 512),
             "wc": wc.astype(ml_dtypes.bfloat16),
             "cbb": cbb, "gib": gib, "cstg": cstgb, "limp": limpb}
        if use_b2:
            m["b2b"] = b2.reshape(2, 128).T.copy()
        in_maps.append(m)

    nc = _build_nc(CS, NSS, NBC, use_b2)
    res = bass_utils.run_bass_kernel_spmd(nc, in_maps, core_ids=list(range(N_CORES)))
    LAST_RESULTS = res

    c_logits = np.zeros((B, C, K), np.float32)
    total = np.zeros((BP, K), np.float64)
    for i in range(N_CORES):
        out = res.results[i]
        cl = out["clout"].reshape(128, NBC * 2, CS, K).transpose(1, 0, 2, 3)
        cl = cl.reshape(BP, CS, K)
        cs, ce = starts[i], starts[i + 1]
        c_logits[:, cs:ce, :] = cl[:B, :ce - cs, :]
        ac = out["acc"].reshape(128, NBC * 2, K).transpose(1, 0, 2).reshape(BP, K)
        total += ac
    logits = (np.log(total[:B] + 1e-16) + bias[None, :]).astype(np.float32)
    return logits, c_logits
